# revision 10
# baseline (speedup 1.0000x reference)
"""DGL-JTNN encoder forward on 8 Trainium2 NeuronCores (Bass/Tile).

Sharding: data-parallel over trees (256 trees -> 32 trees/core), weights
replicated.  All trees share one topology, so per-core work is identical
and the same NEFF runs SPMD on cores 0-7 with per-core input data.

Device layout (per core, all feature-major):
  - features 450 -> 4 chunks of 128 partitions (chunk 3: 66 valid + 62 pad)
  - edge state tile: [128, 30 slots * 256] bf16, slot block = 256 cols =
    {m: 4 chunks x 32 trees | rm: 4 chunks x 32 trees}; slots sorted by
    line-graph topological level so each level's edges are contiguous.
  - per level: DVE copy/add assembles segment sums (s | accum_rm) into a
    stage tile with the same block layout; PE computes the three gates
    with lhsT = weight k-tiles (bias folded in as a K=1 rank-1 matmul,
    which also initializes all 128 PSUM partitions); ACT applies
    sigmoid/tanh; DVE forms m_new/rm and writes them back to the state.
  - final: DVE scatters m into node sums, PE computes relu(W_g [x; m]),
    node-major, and DMAs h out per 128-node block.
"""

import numpy as np
import ml_dtypes

P = 128
H = 450
NCHUNK = 4  # ceil(450/128)
NCORES = 8

_BF = ml_dtypes.bfloat16

_nc_cache = {}


# --------------------------------------------------------------------------
# host-side topology + layout prep
# --------------------------------------------------------------------------

def _topology(edge_src, edge_dst, lg_src, lg_dst, edge_level, n_nodes):
    """Extract the shared per-tree topology and the level schedule."""
    E = len(edge_src)
    npt = 16  # nodes per tree
    # infer nodes-per-tree from edge locality if possible
    n_trees = None
    for cand in (16,):
        if n_nodes % cand == 0:
            n_trees = n_nodes // cand
            npt = cand
            break
    E1 = E // n_trees
    L1 = len(lg_src) // n_trees

    src0 = edge_src[:E1] - 0
    dst0 = edge_dst[:E1] - 0
    lvl0 = edge_level[:E1]
    lgs0 = lg_src[:L1]
    lgd0 = lg_dst[:L1]

    # verify uniform tiling across trees
    node_off = np.repeat(np.arange(n_trees) * npt, E1)
    edge_off = np.repeat(np.arange(n_trees) * E1, L1)
    assert np.array_equal(edge_src, np.tile(src0, n_trees) + node_off), "non-uniform trees"
    assert np.array_equal(edge_dst, np.tile(dst0, n_trees) + node_off), "non-uniform trees"
    assert np.array_equal(lg_src, np.tile(lgs0, n_trees) + edge_off), "non-uniform lg"
    assert np.array_equal(lg_dst, np.tile(lgd0, n_trees) + edge_off), "non-uniform lg"
    assert np.array_equal(edge_level, np.tile(lvl0, n_trees)), "non-uniform levels"

    order = np.argsort(lvl0, kind="stable")  # edge index per slot
    slot_of = np.empty(E1, np.int64)
    slot_of[order] = np.arange(E1)
    nlev = int(lvl0.max()) + 1
    counts = [int((lvl0 == l).sum()) for l in range(nlev)]
    offs = np.concatenate([[0], np.cumsum(counts)]).astype(int)

    # per-slot predecessor slots (in the line graph)
    preds = [sorted(slot_of[lgs0[lgd0 == order[j]]].tolist()) for j in range(E1)]
    # per-node incoming edge slots
    incoming = [sorted(slot_of[np.nonzero(dst0 == u)[0]].tolist()) for u in range(npt)]
    return dict(
        n_trees=n_trees, npt=npt, E1=E1, src0=src0, dst0=dst0, lvl0=lvl0,
        order=order, slot_of=slot_of, nlev=nlev, counts=counts, offs=offs,
        preds=preds, incoming=incoming,
    )


MW = NCHUNK * P  # weight M-stride (450 padded to 512)


def _pack_weight(Wtop, Wbot):
    """[450, 450] halves fp32 -> [128, 8*512] bf16 lhsT layout.

    k-chunk kc in 0..3 covers Wtop rows kc*128.., kc 4..7 covers Wbot.
    Rows/cols beyond 450 in each half are zero (so out partitions 66..127
    of the last m-tile are written with zeros, keeping PSUM finite).
    """
    out = np.zeros((8, P, MW), np.float32)
    for half, Wm in ((0, Wtop), (1, Wbot)):
        for c in range(NCHUNK):
            r0, r1 = c * P, min((c + 1) * P, H)
            out[half * 4 + c, : r1 - r0, :H] = Wm[r0:r1]
    return np.ascontiguousarray(
        out.transpose(1, 0, 2).reshape(P, 8 * MW).astype(_BF))


def _feat_major(rows):
    """[N, 512] -> [128, 4*N] (chunk-major feature layout), keeps dtype."""
    n = rows.shape[0]
    return np.ascontiguousarray(rows.reshape(n, NCHUNK, P).transpose(2, 1, 0)
                                .reshape(P, NCHUNK * n))


def _host_prep(topo, wid, emb, Wz, bz, Wr, Ur, bur, Wh, bh, Wg, bg, n_nodes):
    n_trees, npt, E1 = topo["n_trees"], topo["npt"], topo["E1"]
    tpc = n_trees // NCORES            # trees per core
    nodes_pc = tpc * npt
    epc = tpc * E1

    x = emb[wid].astype(np.float32)                  # [N, 450]
    xpad = np.zeros((n_nodes, NCHUNK * P), np.float32)
    xpad[:, :H] = x
    xpad_bf = xpad.astype(_BF)

    w_z = _pack_weight(Wz[:H], Wz[H:])
    w_h = _pack_weight(Wh[:H], Wh[H:])
    w_u = _pack_weight(Wr, Ur)
    w_g = _pack_weight(Wg[:H], Wg[H:])

    def _brow(b):
        r = np.zeros((1, NCHUNK * P), np.float32)
        r[0, :H] = b
        return r.astype(_BF)

    b_z, b_h, b_u, b_g = _brow(bz), _brow(bh), _brow(bur), _brow(bg)
    ones = np.ones((1, NCHUNK * P), _BF)

    order, src0, dst0 = topo["order"], topo["src0"], topo["dst0"]
    in_maps = []
    for k in range(NCORES):
        base = k * nodes_pc
        # node index for (slot i, tree t): base + t*npt + node0
        tgrid = np.arange(tpc) * npt
        src_nodes = (base + tgrid[None, :] + src0[order][:, None]).reshape(-1)
        dst_nodes = (base + tgrid[None, :] + dst0[order][:, None]).reshape(-1)
        sx = _feat_major(xpad_bf[src_nodes])         # [128, 4*epc]
        dx = _feat_major(xpad_bf[dst_nodes])
        xt = _feat_major(xpad_bf[base:base + nodes_pc])
        in_maps.append({
            "w_z": w_z, "w_h": w_h, "w_u": w_u, "w_g": w_g,
            "b_z": b_z, "b_h": b_h, "b_u": b_u, "b_g": b_g,
            "ones": ones, "sx": sx, "dx": dx, "xt": xt,
        })
    return in_maps, x


# --------------------------------------------------------------------------
# device kernel
# --------------------------------------------------------------------------

def _build_kernel(topo):
    import concourse.bass as bass
    import concourse.bacc as bacc
    import concourse.mybir as mybir
    import concourse.tile as tile

    dt = mybir.dt
    AF = mybir.ActivationFunctionType
    OP = mybir.AluOpType

    n_trees, npt, E1 = topo["n_trees"], topo["npt"], topo["E1"]
    tpc = n_trees // NCORES
    nodes_pc = tpc * npt
    epc = tpc * E1
    nlev, counts, offs = topo["nlev"], topo["counts"], topo["offs"]
    preds, incoming = topo["preds"], topo["incoming"]
    T = tpc                       # trees per core (inner dim of a block)
    BLK = 2 * NCHUNK * T          # state block cols per edge slot (m|rm)
    NB = nodes_pc // P            # node blocks for the final matmul
    # per-gate PSUM layout: chunk mo at free offset mo*256 fp32 — each
    # matmul output must stay inside one 2KB PSUM bank
    assert T * max(counts) <= 256, "level too wide for PSUM chunk stride"

    # Bacc (not plain Bass): its compile() pass moves surplus matmul waits
    # onto LDWEIGHTS and splits >1-wait instructions into event semaphores,
    # which TRN2 codegen requires.
    nc = bacc.Bacc("TRN2", target_bir_lowering=False, debug=False)

    def din(name, shape, dtype=dt.bfloat16):
        return nc.declare_dram_parameter(name, list(shape), dtype, isOutput=False)

    wz_d = din("w_z", (P, 8 * MW))
    wh_d = din("w_h", (P, 8 * MW))
    wu_d = din("w_u", (P, 8 * MW))
    wg_d = din("w_g", (P, 8 * MW))
    bz_d = din("b_z", (1, NCHUNK * P))
    bh_d = din("b_h", (1, NCHUNK * P))
    bu_d = din("b_u", (1, NCHUNK * P))
    bg_d = din("b_g", (1, NCHUNK * P))
    on_d = din("ones", (1, NCHUNK * P))
    sx_d = din("sx", (P, NCHUNK * epc))
    dx_d = din("dx", (P, NCHUNK * epc))
    xt_d = din("xt", (P, NCHUNK * nodes_pc))
    h_d = nc.declare_dram_parameter("hout", [nodes_pc, H], dt.float32, isOutput=True)

    with tile.TileContext(nc) as tc:
        with (
            tc.tile_pool(name="const", bufs=1) as cpool,
            tc.tile_pool(name="stage", bufs=1) as spool,
            tc.tile_pool(name="work", bufs=2) as wpool,
            tc.tile_pool(name="psum", bufs=1, space="PSUM") as ppool,
            tc.tile_pool(name="psumf", bufs=2, space="PSUM") as fpool,
        ):
            # ---- constants / inputs to SBUF ----
            w_z = cpool.tile([P, 8 * MW], dt.bfloat16)
            w_h = cpool.tile([P, 8 * MW], dt.bfloat16)
            w_u = cpool.tile([P, 8 * MW], dt.bfloat16)
            w_g = cpool.tile([P, 8 * MW], dt.bfloat16)
            b_z = cpool.tile([1, NCHUNK * P], dt.bfloat16)
            b_h = cpool.tile([1, NCHUNK * P], dt.bfloat16)
            b_u = cpool.tile([1, NCHUNK * P], dt.bfloat16)
            b_g = cpool.tile([1, NCHUNK * P], dt.bfloat16)
            ones = cpool.tile([1, NCHUNK * P], dt.bfloat16)
            sx = cpool.tile([P, NCHUNK * epc], dt.bfloat16)
            dx = cpool.tile([P, NCHUNK * epc], dt.bfloat16)
            xt = cpool.tile([P, NCHUNK * nodes_pc], dt.bfloat16)

            for t, d in ((w_z, wz_d), (w_h, wh_d), (sx, sx_d), (b_z, bz_d),
                         (b_h, bh_d), (ones, on_d), (w_u, wu_d), (dx, dx_d),
                         (b_u, bu_d), (xt, xt_d), (w_g, wg_d), (b_g, bg_d)):
                nc.sync.dma_start(out=t[:], in_=d[:])

            # views
            sx_v = sx.rearrange("p (c e) -> p c e", c=NCHUNK)
            dx_v = dx.rearrange("p (c e) -> p c e", c=NCHUNK)
            xt_v = xt.rearrange("p (c n) -> p c n", c=NCHUNK)

            # ---- state ----
            state = spool.tile([P, E1 * BLK], dt.bfloat16)
            st_v = state.rearrange("p (e h c t) -> p e h c t", h=2, c=NCHUNK, t=T)

            def lhsT(w, kc, mo):
                return w[:, kc * MW + mo * P: kc * MW + (mo + 1) * P]

            def gate_matmuls(ps, w, brow, static_v, static_off, stage_rhs, N):
                """ps[:, mo, :N] = brow + sum_kc w_kc.T @ rhs_kc  (bf16, fp32 acc)."""
                for mo in range(NCHUNK):
                    nc.tensor.matmul(
                        out=ps[:, mo, :N],
                        lhsT=brow[:, mo * P:(mo + 1) * P],
                        rhs=ones[:, :N],
                        start=True, stop=False,
                    )
                    for kc in range(NCHUNK):
                        nc.tensor.matmul(
                            out=ps[:, mo, :N],
                            lhsT=lhsT(w, kc, mo),
                            rhs=static_v[:, kc, static_off:static_off + N],
                            start=False, stop=False,
                        )
                    for kc in range(NCHUNK):
                        nc.tensor.matmul(
                            out=ps[:, mo, :N],
                            lhsT=lhsT(w, 4 + kc, mo),
                            rhs=stage_rhs(kc),
                            start=False, stop=(kc == NCHUNK - 1),
                        )

            # ---- level loop ----
            for l in range(nlev):
                cl = counts[l]
                off = offs[l]
                N = T * cl

                if l > 0:
                    stg = spool.tile([P, cl * BLK], dt.bfloat16, name=f"stg{l}")
                    stg_v = stg.rearrange("p (e h c t) -> p e h c t",
                                          h=2, c=NCHUNK, t=T)
                    # segment sums: stage block j <- sum of pred state blocks
                    for jj in range(cl):
                        slot = off + jj
                        for r, ps_ in enumerate(preds[slot]):
                            if r == 0:
                                nc.vector.tensor_copy(
                                    out=stg[:, jj * BLK:(jj + 1) * BLK],
                                    in_=state[:, ps_ * BLK:(ps_ + 1) * BLK])
                            else:
                                nc.vector.tensor_add(
                                    out=stg[:, jj * BLK:(jj + 1) * BLK],
                                    in0=stg[:, jj * BLK:(jj + 1) * BLK],
                                    in1=state[:, ps_ * BLK:(ps_ + 1) * BLK])

                zp = ppool.tile([P, NCHUNK, 256], dt.float32, tag="zp", name=f"zp{l}")
                pp = ppool.tile([P, NCHUNK, 256], dt.float32, tag="pp", name=f"pp{l}")
                rp = ppool.tile([P, NCHUNK, 256], dt.float32, tag="rp", name=f"rp{l}")

                if l > 0:
                    zrhs = lambda kc: stg_v[:, :, 0, kc, :]
                    prhs = lambda kc: stg_v[:, :, 1, kc, :]
                    gate_matmuls(zp, w_z, b_z, sx_v, off * T, zrhs, N)
                    gate_matmuls(pp, w_h, b_h, sx_v, off * T, prhs, N)
                else:
                    # src-part + bias only
                    for ps, w, brow in ((zp, w_z, b_z), (pp, w_h, b_h)):
                        for mo in range(NCHUNK):
                            nc.tensor.matmul(out=ps[:, mo, :N],
                                             lhsT=brow[:, mo * P:(mo + 1) * P],
                                             rhs=ones[:, :N], start=True, stop=False)
                            for kc in range(NCHUNK):
                                nc.tensor.matmul(
                                    out=ps[:, mo, :N],
                                    lhsT=lhsT(w, kc, mo),
                                    rhs=sx_v[:, kc, off * T: off * T + N],
                                    start=False, stop=(kc == NCHUNK - 1))

                zt = wpool.tile([P, NCHUNK, N], dt.float32, tag="zt", name=f"zt{l}")
                pt = wpool.tile([P, NCHUNK, N], dt.float32, tag="pt", name=f"pt{l}")
                nc.scalar.activation(out=zt[:], in_=zp[:, :, :N], func=AF.Sigmoid)
                nc.scalar.activation(out=pt[:], in_=pp[:, :, :N], func=AF.Tanh)

                zt_v = zt.rearrange("p c (e t) -> p e c t", t=T)
                pt_v = pt.rearrange("p c (e t) -> p e c t", t=T)
                m_slots = st_v[:, off:off + cl, 0, :, :]
                rm_slots = st_v[:, off:off + cl, 1, :, :]

                if l == 0:
                    # m_new = z * pre_m
                    nc.vector.tensor_mul(out=m_slots, in0=zt_v, in1=pt_v)
                else:
                    s_v = stg_v[:, :, 0, :, :]
                    dtile = wpool.tile([P, NCHUNK, N], dt.float32, tag="dt", name=f"d{l}")
                    d_v = dtile.rearrange("p c (e t) -> p e c t", t=T)
                    # d = pre_m - s ; m_new = s + z*d
                    nc.vector.tensor_sub(out=d_v, in0=pt_v, in1=s_v)
                    nc.vector.tensor_mul(out=dtile[:], in0=zt[:], in1=dtile[:])
                    nc.vector.tensor_add(out=m_slots, in0=d_v, in1=s_v)

                # r = sigmoid(dst_x@Wr + m_new@Ur + bur)
                rrhs = lambda kc: st_v[:, off:off + cl, 0, kc, :]
                gate_matmuls(rp, w_u, b_u, dx_v, off * T, rrhs, N)
                rt = wpool.tile([P, NCHUNK, N], dt.float32, tag="rt", name=f"rt{l}")
                nc.scalar.activation(out=rt[:], in_=rp[:, :, :N], func=AF.Sigmoid)
                rt_v = rt.rearrange("p c (e t) -> p e c t", t=T)
                nc.vector.tensor_mul(out=rm_slots, in0=rt_v, in1=m_slots)

            # ---- final: m_node, h = relu([x, m_node] @ Wg + bg) ----
            mnode = spool.tile([P, NCHUNK * nodes_pc], dt.bfloat16)
            mn_v = mnode.rearrange("p (c t u) -> p c t u", c=NCHUNK, u=npt)
            for u in range(npt):
                for r, e_slot in enumerate(incoming[u]):
                    src_blk = st_v[:, e_slot, 0, :, :]      # [P, 4, T]
                    dst_blk = mn_v[:, :, :, u]              # [P, 4, T]
                    if r == 0:
                        nc.vector.tensor_copy(out=dst_blk, in_=src_blk)
                    else:
                        nc.vector.tensor_add(out=dst_blk, in0=dst_blk, in1=src_blk)

            mn_flat = mnode.rearrange("p (c n) -> p c n", c=NCHUNK)
            for b in range(NB):
                fp = fpool.tile([P, 512], dt.float32, tag="fp", name=f"fp{b}")
                nc.tensor.matmul(out=fp[:, :H], lhsT=ones[:, :P],
                                 rhs=b_g[:, :H], start=True, stop=False)
                for kc in range(8):
                    src = xt_v if kc < NCHUNK else mn_flat
                    nc.tensor.matmul(
                        out=fp[:, :H],
                        lhsT=src[:, kc % NCHUNK, b * P:(b + 1) * P],
                        rhs=w_g[:, kc * MW: kc * MW + H],
                        start=False, stop=(kc == 7))
                h_sb = wpool.tile([P, H], dt.float32, tag="hsb", name=f"hsb{b}")
                nc.scalar.activation(out=h_sb[:], in_=fp[:, :H], func=AF.Relu)
                nc.sync.dma_start(out=h_d[b * P:(b + 1) * P, :], in_=h_sb[:])

    if not nc.is_finalized():
        nc.finalize()
    return nc


# --------------------------------------------------------------------------
# public entry
# --------------------------------------------------------------------------

TRACE = False
LAST_RESULT = None


def kernel(wid, edge_src, edge_dst, lg_src, lg_dst, edge_level, root_ids,
           num_levels, emb, Wz, bz, Wr, Ur, bur, Wh, bh, Wg, bg):
    global LAST_RESULT
    wid = np.asarray(wid)
    edge_src = np.asarray(edge_src); edge_dst = np.asarray(edge_dst)
    lg_src = np.asarray(lg_src); lg_dst = np.asarray(lg_dst)
    edge_level = np.asarray(edge_level); root_ids = np.asarray(root_ids)
    emb = np.asarray(emb, np.float32)
    Wz = np.asarray(Wz, np.float32); bz = np.asarray(bz, np.float32)
    Wr = np.asarray(Wr, np.float32); Ur = np.asarray(Ur, np.float32)
    bur = np.asarray(bur, np.float32)
    Wh = np.asarray(Wh, np.float32); bh = np.asarray(bh, np.float32)
    Wg = np.asarray(Wg, np.float32); bg = np.asarray(bg, np.float32)

    n_nodes = wid.shape[0]
    topo = _topology(edge_src, edge_dst, lg_src, lg_dst, edge_level, n_nodes)
    in_maps, _x = _host_prep(topo, wid, emb, Wz, bz, Wr, Ur, bur,
                             Wh, bh, Wg, bg, n_nodes)

    key = (n_nodes, len(edge_src), len(lg_src),
           tuple(topo["src0"].tolist()), tuple(topo["dst0"].tolist()),
           tuple(topo["lvl0"].tolist()),
           tuple(tuple(p) for p in topo["preds"]))
    if key not in _nc_cache:
        _nc_cache[key] = _build_kernel(topo)
    nc = _nc_cache[key]

    from concourse.bass_utils import run_bass_kernel_spmd
    res = run_bass_kernel_spmd(nc, in_maps, core_ids=list(range(NCORES)),
                               trace=TRACE)
    LAST_RESULT = res

    h = np.concatenate([r["hout"] for r in res.results], axis=0)
    root_vecs = h[root_ids]
    return h, root_vecs


# revision 12
# speedup vs baseline: 1.0753x; 1.0753x over previous
"""DGL-JTNN encoder forward on 8 Trainium2 NeuronCores (Bass/Tile).

Sharding: data-parallel over trees (256 trees -> 32 trees/core), weights
replicated.  All trees share one topology, so per-core work is identical
and the same NEFF runs SPMD on cores 0-7 with per-core input data.

Device layout (per core, all feature-major):
  - features 450 -> 4 chunks of 128 partitions (chunk 3: 66 valid + 62 pad)
  - edge state tile: [128, 30 slots * 256] bf16, slot block = 256 cols =
    {m: 4 chunks x 32 trees | rm: 4 chunks x 32 trees}; slots sorted by
    line-graph topological level so each level's edges are contiguous.
  - per level: DVE copy/add assembles segment sums (s | accum_rm) into a
    stage tile with the same block layout; PE computes the three gates
    with lhsT = weight k-tiles (bias folded in as a K=1 rank-1 matmul,
    which also initializes all 128 PSUM partitions); ACT applies
    sigmoid/tanh; DVE forms m_new/rm and writes them back to the state.
  - final: DVE scatters m into node sums, PE computes relu(W_g [x; m]),
    node-major, and DMAs h out per 128-node block.
"""

import numpy as np
import ml_dtypes

P = 128
H = 450
NCHUNK = 4  # ceil(450/128)
NCORES = 8

_BF = ml_dtypes.bfloat16

_nc_cache = {}


# --------------------------------------------------------------------------
# host-side topology + layout prep
# --------------------------------------------------------------------------

def _topology(edge_src, edge_dst, lg_src, lg_dst, edge_level, n_nodes):
    """Extract the shared per-tree topology and the level schedule."""
    E = len(edge_src)
    npt = 16  # nodes per tree
    # infer nodes-per-tree from edge locality if possible
    n_trees = None
    for cand in (16,):
        if n_nodes % cand == 0:
            n_trees = n_nodes // cand
            npt = cand
            break
    E1 = E // n_trees
    L1 = len(lg_src) // n_trees

    src0 = edge_src[:E1] - 0
    dst0 = edge_dst[:E1] - 0
    lvl0 = edge_level[:E1]
    lgs0 = lg_src[:L1]
    lgd0 = lg_dst[:L1]

    # verify uniform tiling across trees
    node_off = np.repeat(np.arange(n_trees) * npt, E1)
    edge_off = np.repeat(np.arange(n_trees) * E1, L1)
    assert np.array_equal(edge_src, np.tile(src0, n_trees) + node_off), "non-uniform trees"
    assert np.array_equal(edge_dst, np.tile(dst0, n_trees) + node_off), "non-uniform trees"
    assert np.array_equal(lg_src, np.tile(lgs0, n_trees) + edge_off), "non-uniform lg"
    assert np.array_equal(lg_dst, np.tile(lgd0, n_trees) + edge_off), "non-uniform lg"
    assert np.array_equal(edge_level, np.tile(lvl0, n_trees)), "non-uniform levels"

    order = np.argsort(lvl0, kind="stable")  # edge index per slot
    slot_of = np.empty(E1, np.int64)
    slot_of[order] = np.arange(E1)
    nlev = int(lvl0.max()) + 1
    counts = [int((lvl0 == l).sum()) for l in range(nlev)]
    offs = np.concatenate([[0], np.cumsum(counts)]).astype(int)

    # per-slot predecessor slots (in the line graph)
    preds = [sorted(slot_of[lgs0[lgd0 == order[j]]].tolist()) for j in range(E1)]
    # per-node incoming edge slots
    incoming = [sorted(slot_of[np.nonzero(dst0 == u)[0]].tolist()) for u in range(npt)]
    return dict(
        n_trees=n_trees, npt=npt, E1=E1, src0=src0, dst0=dst0, lvl0=lvl0,
        order=order, slot_of=slot_of, nlev=nlev, counts=counts, offs=offs,
        preds=preds, incoming=incoming,
    )


MW = NCHUNK * P  # weight M-stride (450 padded to 512)


def _pack_weight(Wtop, Wbot, bias):
    """[450, 450] halves fp32 -> [128, 8*512] bf16 lhsT layout.

    k-chunk kc in 0..3 covers Wtop rows kc*128.., kc 4..7 covers Wbot.
    Rows/cols beyond 450 in each half are zero (so out partitions 66..127
    of the last m-tile are written with zeros, keeping PSUM finite).
    The bias rides row 127 of k-chunk 3 (the x/src half), paired with the
    constant 1.0 planted in feature column 511 of every x row.
    """
    out = np.zeros((8, P, MW), np.float32)
    for half, Wm in ((0, Wtop), (1, Wbot)):
        for c in range(NCHUNK):
            r0, r1 = c * P, min((c + 1) * P, H)
            out[half * 4 + c, : r1 - r0, :H] = Wm[r0:r1]
    out[3, P - 1, :H] = bias
    return np.ascontiguousarray(
        out.transpose(1, 0, 2).reshape(P, 8 * MW).astype(_BF))


def _feat_major(rows):
    """[N, 512] -> [128, 4*N] (chunk-major feature layout), keeps dtype."""
    n = rows.shape[0]
    return np.ascontiguousarray(rows.reshape(n, NCHUNK, P).transpose(2, 1, 0)
                                .reshape(P, NCHUNK * n))


def _host_prep(topo, wid, emb, Wz, bz, Wr, Ur, bur, Wh, bh, Wg, bg, n_nodes):
    n_trees, npt, E1 = topo["n_trees"], topo["npt"], topo["E1"]
    tpc = n_trees // NCORES            # trees per core
    nodes_pc = tpc * npt
    epc = tpc * E1

    x = emb[wid].astype(np.float32)                  # [N, 450]
    xpad = np.zeros((n_nodes, NCHUNK * P), np.float32)
    xpad[:, :H] = x
    xpad[:, NCHUNK * P - 1] = 1.0   # constant input for the bias row
    xpad_bf = xpad.astype(_BF)

    w_z = _pack_weight(Wz[:H], Wz[H:], bz)
    w_h = _pack_weight(Wh[:H], Wh[H:], bh)
    w_u = _pack_weight(Wr, Ur, bur)
    w_g = _pack_weight(Wg[:H], Wg[H:], bg)

    order, src0, dst0 = topo["order"], topo["src0"], topo["dst0"]
    in_maps = []
    for k in range(NCORES):
        base = k * nodes_pc
        # node index for (slot i, tree t): base + t*npt + node0
        tgrid = np.arange(tpc) * npt
        src_nodes = (base + tgrid[None, :] + src0[order][:, None]).reshape(-1)
        dst_nodes = (base + tgrid[None, :] + dst0[order][:, None]).reshape(-1)
        sx = _feat_major(xpad_bf[src_nodes])         # [128, 4*epc]
        dx = _feat_major(xpad_bf[dst_nodes])
        xt = _feat_major(xpad_bf[base:base + nodes_pc])
        in_maps.append({
            "w_z": w_z, "w_h": w_h, "w_u": w_u, "w_g": w_g,
            "sx": sx, "dx": dx, "xt": xt,
        })
    return in_maps, x


# --------------------------------------------------------------------------
# device kernel
# --------------------------------------------------------------------------

def _build_kernel(topo):
    import concourse.bass as bass
    import concourse.bacc as bacc
    import concourse.mybir as mybir
    import concourse.tile as tile

    dt = mybir.dt
    AF = mybir.ActivationFunctionType
    OP = mybir.AluOpType

    n_trees, npt, E1 = topo["n_trees"], topo["npt"], topo["E1"]
    tpc = n_trees // NCORES
    nodes_pc = tpc * npt
    epc = tpc * E1
    nlev, counts, offs = topo["nlev"], topo["counts"], topo["offs"]
    preds, incoming = topo["preds"], topo["incoming"]
    T = tpc                       # trees per core (inner dim of a block)
    BLK = 2 * NCHUNK * T          # state block cols per edge slot (m|rm)
    NB = nodes_pc // P            # node blocks for the final matmul
    # per-gate PSUM layout: chunk mo at free offset mo*256 fp32 — each
    # matmul output must stay inside one 2KB PSUM bank
    assert T * max(counts) <= 256, "level too wide for PSUM chunk stride"

    # Bacc (not plain Bass): its compile() pass moves surplus matmul waits
    # onto LDWEIGHTS and splits >1-wait instructions into event semaphores,
    # which TRN2 codegen requires.
    nc = bacc.Bacc("TRN2", target_bir_lowering=False, debug=False)

    def din(name, shape, dtype=dt.bfloat16):
        return nc.declare_dram_parameter(name, list(shape), dtype, isOutput=False)

    wz_d = din("w_z", (P, 8 * MW))
    wh_d = din("w_h", (P, 8 * MW))
    wu_d = din("w_u", (P, 8 * MW))
    wg_d = din("w_g", (P, 8 * MW))
    sx_d = din("sx", (P, NCHUNK * epc))
    dx_d = din("dx", (P, NCHUNK * epc))
    xt_d = din("xt", (P, NCHUNK * nodes_pc))
    h_d = nc.declare_dram_parameter("hout", [nodes_pc, H], dt.float32, isOutput=True)

    HB = 480                      # hoist rhs split (<=512 psum fp32 cols)
    hsplits = [(s, min(HB, epc - s)) for s in range(0, epc, HB)]

    with tile.TileContext(nc) as tc:
        with (
            tc.tile_pool(name="const", bufs=1) as cpool,
            tc.tile_pool(name="stage", bufs=1) as spool,
            tc.tile_pool(name="work", bufs=2) as wpool,
            tc.tile_pool(name="psum", bufs=1, space="PSUM") as ppool,
            tc.tile_pool(name="psumf", bufs=2, space="PSUM") as fpool,
        ):
            # ---- inputs to SBUF (sx first: the hoist needs it) ----
            sx = cpool.tile([P, NCHUNK * epc], dt.bfloat16)
            w_z = cpool.tile([P, 8 * MW], dt.bfloat16)
            w_h = cpool.tile([P, 8 * MW], dt.bfloat16)
            w_u = cpool.tile([P, 8 * MW], dt.bfloat16)
            dx = cpool.tile([P, NCHUNK * epc], dt.bfloat16)
            xt = cpool.tile([P, NCHUNK * nodes_pc], dt.bfloat16)
            w_g = cpool.tile([P, 8 * MW], dt.bfloat16)

            for t, d in ((sx, sx_d), (w_z, wz_d), (w_h, wh_d), (w_u, wu_d),
                         (dx, dx_d), (xt, xt_d), (w_g, wg_d)):
                nc.sync.dma_start(out=t[:], in_=d[:])

            sx_v = sx.rearrange("p (c e) -> p c e", c=NCHUNK)
            dx_v = dx.rearrange("p (c e) -> p c e", c=NCHUNK)
            xt_v = xt.rearrange("p (c n) -> p c n", c=NCHUNK)

            # ---- state ----
            state = spool.tile([P, E1 * BLK], dt.bfloat16)
            st_v = state.rearrange("p (e h c t) -> p e h c t", h=2, c=NCHUNK, t=T)

            def lhsT(w, kc, mo):
                return w[:, kc * MW + mo * P: kc * MW + (mo + 1) * P]

            # ---- hoist: A_z = Wz1.T@src_x+bz, A_h = Wh1.T@src_x+bh,
            #             D_r = Wr.T@dst_x+bur   (feature-major, bf16) ----
            az = cpool.tile([P, NCHUNK * epc], dt.bfloat16)
            ah = cpool.tile([P, NCHUNK * epc], dt.bfloat16)
            dr = cpool.tile([P, NCHUNK * epc], dt.bfloat16)
            hoists = ((az, w_z, sx_v), (ah, w_h, sx_v), (dr, w_u, dx_v))
            hidx = 0
            for dst, w, src_v in hoists:
                dst_v = dst.rearrange("p (c e) -> p c e", c=NCHUNK)
                for h0, hw in hsplits:
                    for mo in range(NCHUNK):
                        hp = fpool.tile([P, 512], dt.float32, tag="fp",
                                        name=f"hp{hidx}")
                        for kc in range(NCHUNK):
                            nc.tensor.matmul(
                                out=hp[:, :hw],
                                lhsT=lhsT(w, kc, mo),
                                rhs=src_v[:, kc, h0:h0 + hw],
                                start=(kc == 0), stop=(kc == NCHUNK - 1))
                        # alternate DVE/ACT for the PSUM->SBUF drain
                        if hidx % 2 == 0:
                            nc.vector.tensor_copy(out=dst_v[:, mo, h0:h0 + hw],
                                                  in_=hp[:, :hw])
                        else:
                            nc.scalar.copy(out=dst_v[:, mo, h0:h0 + hw],
                                           in_=hp[:, :hw])
                        hidx += 1
            az_v = az.rearrange("p (c e) -> p c e", c=NCHUNK)
            ah_v = ah.rearrange("p (c e) -> p c e", c=NCHUNK)
            dr_v = dr.rearrange("p (c e) -> p c e", c=NCHUNK)

            def stage_matmuls(ps, w, stage_rhs, N):
                """ps[:, mo, :N] = sum_kc w_{4+kc}.T @ stage_rhs(kc)."""
                for mo in range(NCHUNK):
                    for kc in range(NCHUNK):
                        nc.tensor.matmul(
                            out=ps[:, mo, :N],
                            lhsT=lhsT(w, 4 + kc, mo),
                            rhs=stage_rhs(kc),
                            start=(kc == 0), stop=(kc == NCHUNK - 1))

            def blockify(t2d, cl):
                # [P, 4, N] packed (chunk-major) -> [P, e, c, t] block order
                return t2d.rearrange("p c (e t) -> p e c t", t=T)

            # ---- level loop ----
            for l in range(nlev):
                cl = counts[l]
                off = offs[l]
                N = T * cl
                ecols = slice(off * T, off * T + N)

                if l > 0:
                    stg = spool.tile([P, cl * BLK], dt.bfloat16, name=f"stg{l}")
                    stg_v = stg.rearrange("p (e h c t) -> p e h c t",
                                          h=2, c=NCHUNK, t=T)
                    # segment sums: stage block j <- sum of pred state blocks
                    for jj in range(cl):
                        slot = off + jj
                        for r, ps_ in enumerate(preds[slot]):
                            if r == 0:
                                nc.gpsimd.tensor_copy(
                                    out=stg[:, jj * BLK:(jj + 1) * BLK],
                                    in_=state[:, ps_ * BLK:(ps_ + 1) * BLK])
                            else:
                                nc.vector.tensor_add(
                                    out=stg[:, jj * BLK:(jj + 1) * BLK],
                                    in0=stg[:, jj * BLK:(jj + 1) * BLK],
                                    in1=state[:, ps_ * BLK:(ps_ + 1) * BLK])

                zt = wpool.tile([P, cl, NCHUNK, T], dt.bfloat16, tag="zt",
                                name=f"zt{l}")
                pt = wpool.tile([P, cl, NCHUNK, T], dt.bfloat16, tag="pt",
                                name=f"pt{l}")

                if l > 0:
                    zp = ppool.tile([P, NCHUNK, 256], dt.float32, tag="zp",
                                    name=f"zp{l}")
                    pp = ppool.tile([P, NCHUNK, 256], dt.float32, tag="pp",
                                    name=f"pp{l}")
                    stage_matmuls(zp, w_z, lambda kc: stg_v[:, :, 0, kc, :], N)
                    stage_matmuls(pp, w_h, lambda kc: stg_v[:, :, 1, kc, :], N)
                    zpre = wpool.tile([P, cl, NCHUNK, T], dt.float32, tag="zpre",
                                      name=f"zpre{l}")
                    ppre = wpool.tile([P, cl, NCHUNK, T], dt.float32, tag="ppre",
                                      name=f"ppre{l}")
                    nc.vector.tensor_add(out=zpre[:], in0=blockify(zp[:, :, :N], cl),
                                         in1=blockify(az_v[:, :, ecols], cl))
                    nc.vector.tensor_add(out=ppre[:], in0=blockify(pp[:, :, :N], cl),
                                         in1=blockify(ah_v[:, :, ecols], cl))
                    nc.scalar.activation(out=zt[:], in_=zpre[:], func=AF.Sigmoid)
                    nc.scalar.activation(out=pt[:], in_=ppre[:], func=AF.Tanh)
                else:
                    # level 0: s = accum_rm = 0 -> gates act on A_z/A_h alone
                    nc.scalar.activation(out=zt[:],
                                         in_=blockify(az_v[:, :, ecols], cl),
                                         func=AF.Sigmoid)
                    nc.scalar.activation(out=pt[:],
                                         in_=blockify(ah_v[:, :, ecols], cl),
                                         func=AF.Tanh)

                m_slots = st_v[:, off:off + cl, 0, :, :]
                rm_slots = st_v[:, off:off + cl, 1, :, :]

                if l == 0:
                    # m_new = z * pre_m
                    nc.vector.tensor_mul(out=m_slots, in0=zt[:], in1=pt[:])
                else:
                    s_v = stg_v[:, :, 0, :, :]
                    dtile = wpool.tile([P, cl, NCHUNK, T], dt.bfloat16, tag="dt",
                                       name=f"d{l}")
                    # d = pre_m - s ; m_new = s + z*d
                    nc.vector.tensor_sub(out=dtile[:], in0=pt[:], in1=s_v)
                    nc.vector.tensor_mul(out=dtile[:], in0=zt[:], in1=dtile[:])
                    nc.vector.tensor_add(out=m_slots, in0=dtile[:], in1=s_v)

                # r = sigmoid(D_r + m_new@Ur)
                rp = ppool.tile([P, NCHUNK, 256], dt.float32, tag="rp",
                                name=f"rp{l}")
                stage_matmuls(rp, w_u, lambda kc: st_v[:, off:off + cl, 0, kc, :], N)
                rpre = wpool.tile([P, cl, NCHUNK, T], dt.float32, tag="rpre",
                                  name=f"rpre{l}")
                nc.vector.tensor_add(out=rpre[:], in0=blockify(rp[:, :, :N], cl),
                                     in1=blockify(dr_v[:, :, ecols], cl))
                rt = wpool.tile([P, cl, NCHUNK, T], dt.bfloat16, tag="rt",
                                name=f"rt{l}")
                nc.scalar.activation(out=rt[:], in_=rpre[:], func=AF.Sigmoid)
                nc.vector.tensor_mul(out=rm_slots, in0=rt[:], in1=m_slots)

            # ---- final: m_node, h = relu([x, m_node] @ Wg + bg) ----
            mnode = spool.tile([P, NCHUNK * nodes_pc], dt.bfloat16)
            mn_v = mnode.rearrange("p (c t u) -> p c t u", c=NCHUNK, u=npt)
            for u in range(npt):
                for r, e_slot in enumerate(incoming[u]):
                    src_blk = st_v[:, e_slot, 0, :, :]      # [P, 4, T]
                    dst_blk = mn_v[:, :, :, u]              # [P, 4, T]
                    if r == 0:
                        nc.gpsimd.tensor_copy(out=dst_blk, in_=src_blk)
                    else:
                        nc.vector.tensor_add(out=dst_blk, in0=dst_blk, in1=src_blk)

            mn_flat = mnode.rearrange("p (c n) -> p c n", c=NCHUNK)
            for b in range(NB):
                fp = fpool.tile([P, 512], dt.float32, tag="fp", name=f"fp{b}")
                for kc in range(8):
                    src = xt_v if kc < NCHUNK else mn_flat
                    nc.tensor.matmul(
                        out=fp[:, :H],
                        lhsT=src[:, kc % NCHUNK, b * P:(b + 1) * P],
                        rhs=w_g[:, kc * MW: kc * MW + H],
                        start=(kc == 0), stop=(kc == 7))
                h_sb = wpool.tile([P, H], dt.float32, tag="hsb", name=f"hsb{b}")
                nc.scalar.activation(out=h_sb[:], in_=fp[:, :H], func=AF.Relu)
                nc.sync.dma_start(out=h_d[b * P:(b + 1) * P, :], in_=h_sb[:])

    if not nc.is_finalized():
        nc.finalize()
    return nc


# --------------------------------------------------------------------------
# public entry
# --------------------------------------------------------------------------

TRACE = False
LAST_RESULT = None


def kernel(wid, edge_src, edge_dst, lg_src, lg_dst, edge_level, root_ids,
           num_levels, emb, Wz, bz, Wr, Ur, bur, Wh, bh, Wg, bg):
    global LAST_RESULT
    wid = np.asarray(wid)
    edge_src = np.asarray(edge_src); edge_dst = np.asarray(edge_dst)
    lg_src = np.asarray(lg_src); lg_dst = np.asarray(lg_dst)
    edge_level = np.asarray(edge_level); root_ids = np.asarray(root_ids)
    emb = np.asarray(emb, np.float32)
    Wz = np.asarray(Wz, np.float32); bz = np.asarray(bz, np.float32)
    Wr = np.asarray(Wr, np.float32); Ur = np.asarray(Ur, np.float32)
    bur = np.asarray(bur, np.float32)
    Wh = np.asarray(Wh, np.float32); bh = np.asarray(bh, np.float32)
    Wg = np.asarray(Wg, np.float32); bg = np.asarray(bg, np.float32)

    n_nodes = wid.shape[0]
    topo = _topology(edge_src, edge_dst, lg_src, lg_dst, edge_level, n_nodes)
    in_maps, _x = _host_prep(topo, wid, emb, Wz, bz, Wr, Ur, bur,
                             Wh, bh, Wg, bg, n_nodes)

    key = (n_nodes, len(edge_src), len(lg_src),
           tuple(topo["src0"].tolist()), tuple(topo["dst0"].tolist()),
           tuple(topo["lvl0"].tolist()),
           tuple(tuple(p) for p in topo["preds"]))
    if key not in _nc_cache:
        _nc_cache[key] = _build_kernel(topo)
    nc = _nc_cache[key]

    from concourse.bass_utils import run_bass_kernel_spmd
    res = run_bass_kernel_spmd(nc, in_maps, core_ids=list(range(NCORES)),
                               trace=TRACE)
    LAST_RESULT = res

    h = np.concatenate([r["hout"] for r in res.results], axis=0)
    root_vecs = h[root_ids]
    return h, root_vecs


# revision 14
# speedup vs baseline: 1.4332x; 1.3329x over previous
"""DGL-JTNN encoder forward on 8 Trainium2 NeuronCores (Bass/Tile).

Sharding: data-parallel over trees (256 trees -> 32 trees/core), weights
replicated.  All trees share one topology, so per-core work is identical
and the same NEFF runs SPMD on cores 0-7 with per-core input data.

Device layout (per core, all feature-major):
  - features 450 -> 4 chunks of 128 partitions (chunk 3: 66 valid + 62 pad)
  - edge state tile: [128, 30 slots * 256] bf16, slot block = 256 cols =
    {m: 4 chunks x 32 trees | rm: 4 chunks x 32 trees}; slots sorted by
    line-graph topological level so each level's edges are contiguous.
  - per level: DVE copy/add assembles segment sums (s | accum_rm) into a
    stage tile with the same block layout; PE computes the three gates
    with lhsT = weight k-tiles (bias folded in as a K=1 rank-1 matmul,
    which also initializes all 128 PSUM partitions); ACT applies
    sigmoid/tanh; DVE forms m_new/rm and writes them back to the state.
  - final: DVE scatters m into node sums, PE computes relu(W_g [x; m]),
    node-major, and DMAs h out per 128-node block.
"""

import numpy as np
import ml_dtypes

P = 128
H = 450
NCHUNK = 4  # ceil(450/128)
NCORES = 8

_BF = ml_dtypes.bfloat16

_nc_cache = {}


# --------------------------------------------------------------------------
# host-side topology + layout prep
# --------------------------------------------------------------------------

def _topology(edge_src, edge_dst, lg_src, lg_dst, edge_level, n_nodes):
    """Extract the shared per-tree topology and the level schedule."""
    E = len(edge_src)
    npt = 16  # nodes per tree
    # infer nodes-per-tree from edge locality if possible
    n_trees = None
    for cand in (16,):
        if n_nodes % cand == 0:
            n_trees = n_nodes // cand
            npt = cand
            break
    E1 = E // n_trees
    L1 = len(lg_src) // n_trees

    src0 = edge_src[:E1] - 0
    dst0 = edge_dst[:E1] - 0
    lvl0 = edge_level[:E1]
    lgs0 = lg_src[:L1]
    lgd0 = lg_dst[:L1]

    # verify uniform tiling across trees
    node_off = np.repeat(np.arange(n_trees) * npt, E1)
    edge_off = np.repeat(np.arange(n_trees) * E1, L1)
    assert np.array_equal(edge_src, np.tile(src0, n_trees) + node_off), "non-uniform trees"
    assert np.array_equal(edge_dst, np.tile(dst0, n_trees) + node_off), "non-uniform trees"
    assert np.array_equal(lg_src, np.tile(lgs0, n_trees) + edge_off), "non-uniform lg"
    assert np.array_equal(lg_dst, np.tile(lgd0, n_trees) + edge_off), "non-uniform lg"
    assert np.array_equal(edge_level, np.tile(lvl0, n_trees)), "non-uniform levels"

    order = np.argsort(lvl0, kind="stable")  # edge index per slot
    slot_of = np.empty(E1, np.int64)
    slot_of[order] = np.arange(E1)
    nlev = int(lvl0.max()) + 1
    counts = [int((lvl0 == l).sum()) for l in range(nlev)]
    offs = np.concatenate([[0], np.cumsum(counts)]).astype(int)

    # per-slot predecessor slots (in the line graph)
    preds = [sorted(slot_of[lgs0[lgd0 == order[j]]].tolist()) for j in range(E1)]
    # per-node incoming edge slots
    incoming = [sorted(slot_of[np.nonzero(dst0 == u)[0]].tolist()) for u in range(npt)]
    return dict(
        n_trees=n_trees, npt=npt, E1=E1, src0=src0, dst0=dst0, lvl0=lvl0,
        order=order, slot_of=slot_of, nlev=nlev, counts=counts, offs=offs,
        preds=preds, incoming=incoming,
    )


MW = NCHUNK * P  # weight M-stride (450 padded to 512)


def _pack_weight(Wtop, Wbot, bias):
    """[450, 450] halves fp32 -> [128, 8*512] bf16 lhsT layout.

    k-chunk kc in 0..3 covers Wtop rows kc*128.., kc 4..7 covers Wbot.
    Rows/cols beyond 450 in each half are zero (so out partitions 66..127
    of the last m-tile are written with zeros, keeping PSUM finite).
    The bias rides row 127 of k-chunk 3 (the x/src half), paired with the
    constant 1.0 planted in feature column 511 of every x row.
    """
    out = np.zeros((8, P, MW), np.float32)
    for half, Wm in ((0, Wtop), (1, Wbot)):
        for c in range(NCHUNK):
            r0, r1 = c * P, min((c + 1) * P, H)
            out[half * 4 + c, : r1 - r0, :H] = Wm[r0:r1]
    out[3, P - 1, :H] = bias
    return np.ascontiguousarray(
        out.transpose(1, 0, 2).reshape(P, 8 * MW).astype(_BF))


def _feat_major(rows):
    """[N, 512] -> [128, 4*N] (chunk-major feature layout), keeps dtype."""
    n = rows.shape[0]
    return np.ascontiguousarray(rows.reshape(n, NCHUNK, P).transpose(2, 1, 0)
                                .reshape(P, NCHUNK * n))


def _host_prep(topo, wid, emb, Wz, bz, Wr, Ur, bur, Wh, bh, Wg, bg, n_nodes):
    n_trees, npt, E1 = topo["n_trees"], topo["npt"], topo["E1"]
    tpc = n_trees // NCORES            # trees per core
    nodes_pc = tpc * npt
    epc = tpc * E1

    x = emb[wid].astype(np.float32)                  # [N, 450]
    xpad = np.zeros((n_nodes, NCHUNK * P), np.float32)
    xpad[:, :H] = x
    xpad[:, NCHUNK * P - 1] = 1.0   # constant input for the bias row
    xpad_bf = xpad.astype(_BF)

    w_z = _pack_weight(Wz[:H], Wz[H:], bz)
    w_h = _pack_weight(Wh[:H], Wh[H:], bh)
    w_u = _pack_weight(Wr, Ur, bur)
    w_g = _pack_weight(Wg[:H], Wg[H:], bg)

    order, src0, dst0 = topo["order"], topo["src0"], topo["dst0"]
    in_maps = []
    for k in range(NCORES):
        base = k * nodes_pc
        # node index for (slot i, tree t): base + t*npt + node0
        tgrid = np.arange(tpc) * npt
        src_nodes = (base + tgrid[None, :] + src0[order][:, None]).reshape(-1)
        dst_nodes = (base + tgrid[None, :] + dst0[order][:, None]).reshape(-1)
        sx = _feat_major(xpad_bf[src_nodes])         # [128, 4*epc]
        dx = _feat_major(xpad_bf[dst_nodes])
        xt = _feat_major(xpad_bf[base:base + nodes_pc])
        in_maps.append({
            "w_z": w_z, "w_h": w_h, "w_u": w_u, "w_g": w_g,
            "ident": np.eye(P, dtype=_BF),
            "sx": sx, "dx": dx, "xt": xt,
        })
    return in_maps, x


# --------------------------------------------------------------------------
# device kernel
# --------------------------------------------------------------------------

def _build_kernel(topo):
    import concourse.bass as bass
    import concourse.bacc as bacc
    import concourse.mybir as mybir
    import concourse.tile as tile

    dt = mybir.dt
    AF = mybir.ActivationFunctionType
    OP = mybir.AluOpType

    n_trees, npt, E1 = topo["n_trees"], topo["npt"], topo["E1"]
    tpc = n_trees // NCORES
    nodes_pc = tpc * npt
    epc = tpc * E1
    nlev, counts, offs = topo["nlev"], topo["counts"], topo["offs"]
    preds, incoming = topo["preds"], topo["incoming"]
    T = tpc                       # trees per core (inner dim of a block)
    BLK = 2 * NCHUNK * T          # state block cols per edge slot (m|rm)
    NB = nodes_pc // P            # node blocks for the final matmul
    # per-gate PSUM layout: chunk mo at free offset mo*256 fp32 — each
    # matmul output must stay inside one 2KB PSUM bank
    assert T * max(counts) <= 256, "level too wide for PSUM chunk stride"

    # Bacc (not plain Bass): its compile() pass moves surplus matmul waits
    # onto LDWEIGHTS and splits >1-wait instructions into event semaphores,
    # which TRN2 codegen requires.
    nc = bacc.Bacc("TRN2", target_bir_lowering=False, debug=False)

    def din(name, shape, dtype=dt.bfloat16):
        return nc.declare_dram_parameter(name, list(shape), dtype, isOutput=False)

    wz_d = din("w_z", (P, 8 * MW))
    wh_d = din("w_h", (P, 8 * MW))
    wu_d = din("w_u", (P, 8 * MW))
    wg_d = din("w_g", (P, 8 * MW))
    id_d = din("ident", (P, P))
    sx_d = din("sx", (P, NCHUNK * epc))
    dx_d = din("dx", (P, NCHUNK * epc))
    xt_d = din("xt", (P, NCHUNK * nodes_pc))
    h_d = nc.declare_dram_parameter("hout", [nodes_pc, H], dt.float32, isOutput=True)

    HB = 480                      # hoist rhs split (<=512 psum fp32 cols)
    hsplits = [(s, min(HB, epc - s)) for s in range(0, epc, HB)]

    with tile.TileContext(nc) as tc:
        with (
            tc.tile_pool(name="const", bufs=1) as cpool,
            tc.tile_pool(name="stage", bufs=1) as spool,
            tc.tile_pool(name="work", bufs=2) as wpool,
            tc.tile_pool(name="psum", bufs=1, space="PSUM") as ppool,
            tc.tile_pool(name="psumf", bufs=2, space="PSUM") as fpool,
        ):
            # ---- inputs to SBUF (sx first: the hoist needs it) ----
            ident = cpool.tile([P, P], dt.bfloat16)
            sx = cpool.tile([P, NCHUNK * epc], dt.bfloat16)
            w_z = cpool.tile([P, 8 * MW], dt.bfloat16)
            w_h = cpool.tile([P, 8 * MW], dt.bfloat16)
            w_u = cpool.tile([P, 8 * MW], dt.bfloat16)
            dx = cpool.tile([P, NCHUNK * epc], dt.bfloat16)
            xt = cpool.tile([P, NCHUNK * nodes_pc], dt.bfloat16)
            w_g = cpool.tile([P, 8 * MW], dt.bfloat16)

            for t, d in ((ident, id_d), (sx, sx_d), (w_z, wz_d), (w_h, wh_d),
                         (w_u, wu_d), (dx, dx_d), (xt, xt_d), (w_g, wg_d)):
                nc.sync.dma_start(out=t[:], in_=d[:])

            sx_v = sx.rearrange("p (c e) -> p c e", c=NCHUNK)
            dx_v = dx.rearrange("p (c e) -> p c e", c=NCHUNK)
            xt_v = xt.rearrange("p (c n) -> p c n", c=NCHUNK)

            # ---- state ----
            state = spool.tile([P, E1 * BLK], dt.bfloat16)
            st_v = state.rearrange("p (e h c t) -> p e h c t", h=2, c=NCHUNK, t=T)

            def lhsT(w, kc, mo):
                return w[:, kc * MW + mo * P: kc * MW + (mo + 1) * P]

            # ---- hoist: A_z = Wz1.T@src_x+bz, A_h = Wh1.T@src_x+bh,
            #             D_r = Wr.T@dst_x+bur   (feature-major, bf16) ----
            az = cpool.tile([P, NCHUNK * epc], dt.bfloat16)
            ah = cpool.tile([P, NCHUNK * epc], dt.bfloat16)
            dr = cpool.tile([P, NCHUNK * epc], dt.bfloat16)
            hoists = ((az, w_z, sx_v), (ah, w_h, sx_v), (dr, w_u, dx_v))
            hidx = 0
            for dst, w, src_v in hoists:
                dst_v = dst.rearrange("p (c e) -> p c e", c=NCHUNK)
                for h0, hw in hsplits:
                    for mo in range(NCHUNK):
                        hp = fpool.tile([P, 512], dt.float32, tag="fp",
                                        name=f"hp{hidx}")
                        for kc in range(NCHUNK):
                            nc.tensor.matmul(
                                out=hp[:, :hw],
                                lhsT=lhsT(w, kc, mo),
                                rhs=src_v[:, kc, h0:h0 + hw],
                                start=(kc == 0), stop=(kc == NCHUNK - 1))
                        # alternate DVE/ACT for the PSUM->SBUF drain
                        if hidx % 2 == 0:
                            nc.vector.tensor_copy(out=dst_v[:, mo, h0:h0 + hw],
                                                  in_=hp[:, :hw])
                        else:
                            nc.scalar.copy(out=dst_v[:, mo, h0:h0 + hw],
                                           in_=hp[:, :hw])
                        hidx += 1
            az_v = az.rearrange("p (c e) -> p c e", c=NCHUNK)
            ah_v = ah.rearrange("p (c e) -> p c e", c=NCHUNK)
            dr_v = dr.rearrange("p (c e) -> p c e", c=NCHUNK)

            def stage_matmuls(ps, w, base_v, ecols, stage_rhs, N):
                """ps[:, mo, :N] = base (identity preload) + sum_kc w.T@stage."""
                for mo in range(NCHUNK):
                    nc.tensor.matmul(
                        out=ps[:, mo, :N], lhsT=ident[:],
                        rhs=base_v[:, mo, ecols], start=True, stop=False)
                    for kc in range(NCHUNK):
                        nc.tensor.matmul(
                            out=ps[:, mo, :N],
                            lhsT=lhsT(w, 4 + kc, mo),
                            rhs=stage_rhs(kc),
                            start=False, stop=(kc == NCHUNK - 1))

            def blockify(t2d, cl):
                # [P, 4, N] packed (chunk-major) -> [P, e, c, t] block order
                return t2d.rearrange("p c (e t) -> p e c t", t=T)

            # ---- level loop ----
            for l in range(nlev):
                cl = counts[l]
                off = offs[l]
                N = T * cl
                ecols = slice(off * T, off * T + N)

                if l > 0:
                    stg = spool.tile([P, cl * BLK], dt.bfloat16, name=f"stg{l}")
                    stg_v = stg.rearrange("p (e h c t) -> p e h c t",
                                          h=2, c=NCHUNK, t=T)
                    # segment sums per half (hf=0: m, hf=1: rm) so the z-gate
                    # of this level doesn't wait for the preds' rm writes
                    HB2 = BLK // 2
                    for hf in (0, 1):
                        for jj in range(cl):
                            slot = off + jj
                            pl = preds[slot]
                            dst = stg[:, jj * BLK + hf * HB2:
                                      jj * BLK + (hf + 1) * HB2]
                            blk = lambda e: state[:, e * BLK + hf * HB2:
                                                  e * BLK + (hf + 1) * HB2]
                            if len(pl) == 1:
                                nc.vector.tensor_copy(out=dst, in_=blk(pl[0]))
                            else:
                                nc.vector.tensor_add(out=dst, in0=blk(pl[0]),
                                                     in1=blk(pl[1]))
                                for ps_ in pl[2:]:
                                    nc.vector.tensor_add(out=dst, in0=dst,
                                                         in1=blk(ps_))

                zt = wpool.tile([P, cl, NCHUNK, T], dt.bfloat16, tag="zt",
                                name=f"zt{l}")
                pt = wpool.tile([P, cl, NCHUNK, T], dt.bfloat16, tag="pt",
                                name=f"pt{l}")

                if l > 0:
                    zp = ppool.tile([P, NCHUNK, 256], dt.float32, tag="zp",
                                    name=f"zp{l}")
                    pp = ppool.tile([P, NCHUNK, 256], dt.float32, tag="pp",
                                    name=f"pp{l}")
                    stage_matmuls(zp, w_z, az_v, ecols,
                                  lambda kc: stg_v[:, :, 0, kc, :], N)
                    stage_matmuls(pp, w_h, ah_v, ecols,
                                  lambda kc: stg_v[:, :, 1, kc, :], N)
                    nc.scalar.activation(out=zt[:], in_=blockify(zp[:, :, :N], cl),
                                         func=AF.Sigmoid)
                    nc.scalar.activation(out=pt[:], in_=blockify(pp[:, :, :N], cl),
                                         func=AF.Tanh)
                else:
                    # level 0: s = accum_rm = 0 -> gates act on A_z/A_h alone
                    nc.scalar.activation(out=zt[:],
                                         in_=blockify(az_v[:, :, ecols], cl),
                                         func=AF.Sigmoid)
                    nc.scalar.activation(out=pt[:],
                                         in_=blockify(ah_v[:, :, ecols], cl),
                                         func=AF.Tanh)

                m_slots = st_v[:, off:off + cl, 0, :, :]
                rm_slots = st_v[:, off:off + cl, 1, :, :]

                if l == 0:
                    # m_new = z * pre_m
                    nc.vector.tensor_mul(out=m_slots, in0=zt[:], in1=pt[:])
                else:
                    s_v = stg_v[:, :, 0, :, :]
                    dtile = wpool.tile([P, cl, NCHUNK, T], dt.bfloat16, tag="dt",
                                       name=f"d{l}")
                    # d = pre_m - s ; m_new = s + z*d
                    nc.vector.tensor_sub(out=dtile[:], in0=pt[:], in1=s_v)
                    nc.vector.tensor_mul(out=dtile[:], in0=zt[:], in1=dtile[:])
                    nc.vector.tensor_add(out=m_slots, in0=dtile[:], in1=s_v)

                # r = sigmoid(D_r + m_new@Ur)
                rp = ppool.tile([P, NCHUNK, 256], dt.float32, tag="rp",
                                name=f"rp{l}")
                stage_matmuls(rp, w_u, dr_v, ecols,
                              lambda kc: st_v[:, off:off + cl, 0, kc, :], N)
                rt = wpool.tile([P, cl, NCHUNK, T], dt.bfloat16, tag="rt",
                                name=f"rt{l}")
                nc.scalar.activation(out=rt[:], in_=blockify(rp[:, :, :N], cl),
                                     func=AF.Sigmoid)
                nc.vector.tensor_mul(out=rm_slots, in0=rt[:], in1=m_slots)

            # ---- final: m_node, h = relu([x, m_node] @ Wg + bg) ----
            mnode = spool.tile([P, NCHUNK * nodes_pc], dt.bfloat16)
            mn_v = mnode.rearrange("p (c t u) -> p c t u", c=NCHUNK, u=npt)
            for u in range(npt):
                for r, e_slot in enumerate(incoming[u]):
                    src_blk = st_v[:, e_slot, 0, :, :]      # [P, 4, T]
                    dst_blk = mn_v[:, :, :, u]              # [P, 4, T]
                    if r == 0:
                        nc.vector.tensor_copy(out=dst_blk, in_=src_blk)
                    else:
                        nc.vector.tensor_add(out=dst_blk, in0=dst_blk, in1=src_blk)

            mn_flat = mnode.rearrange("p (c n) -> p c n", c=NCHUNK)
            for b in range(NB):
                fp = fpool.tile([P, 512], dt.float32, tag="fp", name=f"fp{b}")
                for kc in range(8):
                    src = xt_v if kc < NCHUNK else mn_flat
                    nc.tensor.matmul(
                        out=fp[:, :H],
                        lhsT=src[:, kc % NCHUNK, b * P:(b + 1) * P],
                        rhs=w_g[:, kc * MW: kc * MW + H],
                        start=(kc == 0), stop=(kc == 7))
                h_sb = wpool.tile([P, H], dt.float32, tag="hsb", name=f"hsb{b}")
                nc.scalar.activation(out=h_sb[:], in_=fp[:, :H], func=AF.Relu)
                nc.sync.dma_start(out=h_d[b * P:(b + 1) * P, :], in_=h_sb[:])

    if not nc.is_finalized():
        nc.finalize()
    return nc


# --------------------------------------------------------------------------
# public entry
# --------------------------------------------------------------------------

TRACE = False
LAST_RESULT = None


def kernel(wid, edge_src, edge_dst, lg_src, lg_dst, edge_level, root_ids,
           num_levels, emb, Wz, bz, Wr, Ur, bur, Wh, bh, Wg, bg):
    global LAST_RESULT
    wid = np.asarray(wid)
    edge_src = np.asarray(edge_src); edge_dst = np.asarray(edge_dst)
    lg_src = np.asarray(lg_src); lg_dst = np.asarray(lg_dst)
    edge_level = np.asarray(edge_level); root_ids = np.asarray(root_ids)
    emb = np.asarray(emb, np.float32)
    Wz = np.asarray(Wz, np.float32); bz = np.asarray(bz, np.float32)
    Wr = np.asarray(Wr, np.float32); Ur = np.asarray(Ur, np.float32)
    bur = np.asarray(bur, np.float32)
    Wh = np.asarray(Wh, np.float32); bh = np.asarray(bh, np.float32)
    Wg = np.asarray(Wg, np.float32); bg = np.asarray(bg, np.float32)

    n_nodes = wid.shape[0]
    topo = _topology(edge_src, edge_dst, lg_src, lg_dst, edge_level, n_nodes)
    in_maps, _x = _host_prep(topo, wid, emb, Wz, bz, Wr, Ur, bur,
                             Wh, bh, Wg, bg, n_nodes)

    key = (n_nodes, len(edge_src), len(lg_src),
           tuple(topo["src0"].tolist()), tuple(topo["dst0"].tolist()),
           tuple(topo["lvl0"].tolist()),
           tuple(tuple(p) for p in topo["preds"]))
    if key not in _nc_cache:
        _nc_cache[key] = _build_kernel(topo)
    nc = _nc_cache[key]

    from concourse.bass_utils import run_bass_kernel_spmd
    res = run_bass_kernel_spmd(nc, in_maps, core_ids=list(range(NCORES)),
                               trace=TRACE)
    LAST_RESULT = res

    h = np.concatenate([r["hout"] for r in res.results], axis=0)
    root_vecs = h[root_ids]
    return h, root_vecs
